# revision 1
# baseline (speedup 1.0000x reference)
"""DCGRU cell on 8 Trainium2 NeuronCores.

Strategy (data-parallel over batch B=64 -> 8 per core):
  - Sparse supports are densified on host into S^T [2048, 2048] bf16 and
    streamed column-batched from HBM as matmul stationary operands; the
    diffusion spmm runs as dense bf16 matmul (fp32 PSUM accumulate).
  - Activations live node-major ("natural") [n, (b,f)] in bf16; the
    Chebyshev recursion x2 = 2 S x1 - x0 is folded as xs2' = S x1 -
    0.5 x0 with W2' = 2 W2 (host pre-scales the k=2 W rows).
  - The projection contraction (over features f and matrix index m)
    needs feature-major operands, so each diffusion output is
    PE-transposed into bf16 tiles xs^T. W is host-reordered so state
    features contract as K=64 groups (W rows duplicated at partition
    base 64 so lhsT/rhs bases match) and the 5x2 input features as one
    K=10 group gathered into partition-base-aligned packed tiles.
  - Gate output stays feature-major: r is transposed back and multiplied
    into the natural x0 in place (building the candidate input); u and c
    take a DRAM round-trip; the final GRU combine runs in natural layout
    against an fp32 state re-read, and the output is written as
    [n, (b, u)] fp32 which the host untransposes.
"""

import numpy as np

import concourse.bass as bass
from concourse import bacc
import concourse.mybir as mybir
import concourse.tile as tile
from concourse.bass_utils import run_bass_kernel_spmd
from concourse.masks import make_identity

N = 2048            # nodes
B = 64              # global batch
BL = 8              # batch per core
NCORES = 8
D_IN = 2
U = 64              # hidden units
M = 5               # 1 + 2 supports * K
F = D_IN + U        # 66
NB = N // 128       # 16 node blocks
SC = BL * U         # 512 state cols in natural layout
IC = BL * D_IN      # 16 input cols
CW = SC + IC        # 528 total natural cols
PK = M * D_IN       # 10 packed input rows per batch

F32 = mybir.dt.float32
BF16 = mybir.dt.bfloat16


def _build_nc():
    nc = bacc.Bacc(None, target_bir_lowering=False)

    x0d = nc.declare_dram_parameter("x0", [N, CW], BF16, isOutput=False)
    stfd = nc.declare_dram_parameter("statef", [N, SC], F32, isOutput=False)
    satd = nc.declare_dram_parameter("sat", [N, N], BF16, isOutput=False)
    sbtd = nc.declare_dram_parameter("sbt", [N, N], BF16, isOutput=False)
    wgsd = nc.declare_dram_parameter("wgs", [128, M * 128], BF16, isOutput=False)
    wgid = nc.declare_dram_parameter("wgi", [128, 128], BF16, isOutput=False)
    wcsd = nc.declare_dram_parameter("wcs", [128, M * U], BF16, isOutput=False)
    wcid = nc.declare_dram_parameter("wci", [128, U], BF16, isOutput=False)
    bgd = nc.declare_dram_parameter("bg", [128, 1], F32, isOutput=False)
    bcd = nc.declare_dram_parameter("bc", [U, 1], F32, isOutput=False)
    outd = nc.declare_dram_parameter("out", [N, SC], F32, isOutput=True)
    ubufd = nc.dram_tensor("ubuf", [128, BL * 1024], F32)
    cbufd = nc.dram_tensor("cbuf", [128, BL * 1024], F32)

    with tile.TileContext(nc) as tc:
        _emit(nc, tc, x0d, stfd, satd, sbtd, wgsd, wgid, wcsd, wcid, bgd,
              bcd, outd, ubufd, cbufd)
    nc.compile()
    return nc


def _emit(nc, tc, x0d, stfd, satd, sbtd, wgsd, wgid, wcsd, wcid, bgd, bcd,
          outd, ubufd, cbufd):
    from contextlib import ExitStack
    ctx = ExitStack()
    with ctx:
        consts = ctx.enter_context(tc.tile_pool(name="consts", bufs=1))
        nat = ctx.enter_context(tc.tile_pool(name="nat", bufs=1))
        xst = ctx.enter_context(tc.tile_pool(name="xst", bufs=1))
        x2p = ctx.enter_context(tc.tile_pool(name="x2p", bufs=3))
        spool = ctx.enter_context(tc.tile_pool(name="spool", bufs=3))
        small = ctx.enter_context(tc.tile_pool(name="small", bufs=2))
        psum = ctx.enter_context(tc.tile_pool(name="psum", bufs=8, space="PSUM"))

        ident = consts.tile([128, 128], F32)
        make_identity(nc, ident[:])
        identb = consts.tile([128, 128], BF16)
        nc.vector.tensor_copy(identb[:], ident[:])

        wgs = consts.tile([128, M * 128], BF16)
        wgi = consts.tile([128, 128], BF16)
        wcs = consts.tile([128, M * U], BF16)
        wci = consts.tile([128, U], BF16)
        bg = consts.tile([128, 1], F32)
        bc = consts.tile([U, 1], F32)
        for dst, src in ((wgs, wgsd), (wgi, wgid), (wcs, wcsd), (wci, wcid),
                         (bg, bgd), (bc, bcd)):
            nc.sync.dma_start(dst[:], src[:])

        # natural-layout activations (bf16): block i at cols i*CW
        x0sb = nat.tile([128, NB * CW], BF16, tag="x0")
        x1sb = nat.tile([128, NB * CW], BF16, tag="x1")
        x0dv = x0d.rearrange("(t p) c -> t p c", p=128)
        for i in range(NB):
            nc.sync.dma_start(x0sb[:, i * CW:(i + 1) * CW], x0dv[i])

        # xs^T state parts, bf16: block (m, j) = batches {2j, 2j+1}, all n
        xsts = xst.tile([128, M * 4 * N], BF16, tag="xsts")
        # input-feature diffusion, natural gather: cols b*32 + m*D_IN + fi
        xicat = xst.tile([128, NB * 256], BF16, tag="xicat")
        nc.vector.memset(xicat[:], 0.0)
        # packed input-feature rhs: tile t, batch b=3t+k at rows 32*k
        xpk = [xst.tile([128, N], BF16, tag=f"xpk{t}", name=f"xpk{t}")
               for t in range(3)]

        def xst_s(m, j):
            return xsts[:, (m * 4 + j) * N:(m * 4 + j + 1) * N]

        def transpose_to_xst(m, i, src_ap, with_input):
            """src_ap: natural bf16 block [128, CW-ish]; writes xs^T."""
            for j in range(4):
                pt = psum.tile([128, 128], BF16, tag="ps")
                nc.tensor.transpose(
                    pt[:], src_ap[:, j * 128:(j + 1) * 128], identb[:])
                nc.vector.tensor_copy(
                    xst_s(m, j)[:, i * 128:(i + 1) * 128], pt[:])
            if with_input:
                # gather input cols into xicat (b,f)-strided -> (b,m,f)
                src3 = src_ap[:, SC:SC + IC].rearrange("p (b f) -> p b f", b=BL)
                dst3 = xicat[:, i * 256:(i + 1) * 256].rearrange(
                    "p (b r) -> p b r", r=32)[:, :, m * D_IN:(m + 1) * D_IN]
                nc.vector.tensor_copy(dst3, src3)

        def spmm(std, xsrc, chunks, dest_cb):
            """Y = S @ X (bf16). Per row-block: one column-batched S DMA,
            then K-accumulated matmuls; dest_cb(i, psum_list) consumes."""
            stdv = std.rearrange("(j p) c -> p j c", p=128)
            for i in range(NB):
                sc = spool.tile([128, NB * 128], BF16, tag="sc")
                nc.sync.dma_start(
                    sc[:].rearrange("p (j c) -> p j c", j=NB),
                    stdv[:, :, i * 128:(i + 1) * 128])
                pts = [psum.tile([128, c1 - c0], F32, tag="ps",
                                 name=f"pmm{i}_{c0}")
                       for (c0, c1) in chunks]
                for j in range(NB):
                    for ci, (c0, c1) in enumerate(chunks):
                        nc.tensor.matmul(
                            pts[ci][:], sc[:, j * 128:(j + 1) * 128],
                            xsrc[:, j * CW + c0:j * CW + c1],
                            start=(j == 0), stop=(j == NB - 1))
                dest_cb(i, pts)

        GCH = [(0, 512), (512, 528)]
        CCH = [(0, 512)]

        def dconv(xnat, x1nat, is_gate):
            """Emit one diffusion-conv's spmm + transpose stages."""
            chunks = GCH if is_gate else CCH
            wid = SC + (IC if is_gate else 0)
            for i in range(NB):
                transpose_to_xst(0, i, xnat[:, i * CW:i * CW + CW], is_gate)
            for sup, std in ((0, satd), (1, sbtd)):
                m1, m2 = 1 + 2 * sup, 2 + 2 * sup

                def x1_sink(i, pts):
                    for pt, (c0, c1) in zip(pts, chunks):
                        nc.vector.tensor_copy(
                            x1nat[:, i * CW + c0:i * CW + c1], pt[:])
                    transpose_to_xst(m1, i, x1nat[:, i * CW:i * CW + CW],
                                     is_gate)

                spmm(std, xnat, chunks, x1_sink)

                # x2' = S x1 - 0.5 x0  (W of the k=2 terms pre-doubled)
                def x2_sink(i, pts):
                    blk = x2p.tile([128, CW], BF16, tag="x2")
                    for pt, (c0, c1) in zip(pts, chunks):
                        nc.vector.scalar_tensor_tensor(
                            blk[:, c0:c1],
                            xnat[:, i * CW + c0:i * CW + c1],
                            -0.5, pt[:],
                            mybir.AluOpType.mult, mybir.AluOpType.add)
                    transpose_to_xst(m2, i, blk[:, 0:wid], is_gate)

                spmm(std, x1nat, chunks, x2_sink)

        def finalize_inputs():
            for i in range(NB):
                for t in range(3):
                    w = 96 if t < 2 else 64
                    pt = psum.tile([w, 128], BF16, tag="ps", name=f"pfin{t}")
                    nc.tensor.transpose(
                        pt[:], xicat[:, i * 256 + t * 96:i * 256 + t * 96 + w],
                        identb[:])
                    nc.vector.tensor_copy(xpk[t][:w, i * 128:(i + 1) * 128],
                                          pt[:])

        def w_stage(is_gate):
            """Projection + activation. Gate: sigmoid -> r (into x0sb),
            u (to DRAM). Cand: tanh -> c (to DRAM)."""
            ws, wi, O = (wgs, wgi, 128) if is_gate else (wcs, wci, U)
            for b in range(BL):
                t, k = b // 3, b % 3
                for c in range(4):  # n-chunks of 512
                    pt = psum.tile([O, 512], F32, tag="ps")
                    bp = (b % 2) * U
                    for m in range(M):
                        rs = xst_s(m, b // 2)[bp:bp + U, c * 512:(c + 1) * 512]
                        nc.tensor.matmul(pt[:], ws[bp:bp + U, m * O:(m + 1) * O],
                                         rs, start=(m == 0), stop=False)
                    ri = xpk[t][32 * k:32 * k + PK, c * 512:(c + 1) * 512]
                    nc.tensor.matmul(pt[:], wi[32 * k:32 * k + PK, :O], ri,
                                     start=False, stop=True)
                    h = c // 2
                    cols = slice(b * 1024 + 512 * (c % 2),
                                 b * 1024 + 512 * (c % 2) + 512)
                    if is_gate:
                        rb = small.tile([U, 512], F32, tag="rb")
                        nc.scalar.activation(rb[:], pt[:U, :],
                                             mybir.ActivationFunctionType.Sigmoid,
                                             bias=bg[:U, :])
                        ub = small.tile([U, 512], F32, tag="ub")
                        nc.scalar.activation(ub[:], pt[U:128, :],
                                             mybir.ActivationFunctionType.Sigmoid,
                                             bias=bg[U:128, :])
                        nc.sync.dma_start(ubufd[64 * h:64 * h + 64, cols], ub[:])
                        # r^T into x0 state cols (candidate input, in place)
                        rpt = psum.tile([128, 256], F32, tag="ps")
                        for j in range(4):
                            nc.tensor.transpose(
                                rpt[:, j * U:(j + 1) * U],
                                rb[:, j * 128:(j + 1) * 128], ident[:U, :U])
                        xv = x0sb[:].rearrange("p (i c) -> p i c", c=CW)[
                            :, 4 * c:4 * c + 4, b * U:(b + 1) * U]
                        nc.vector.tensor_mul(
                            xv, xv,
                            rpt[:].rearrange("p (i o) -> p i o", o=U))
                    else:
                        cb = small.tile([U, 512], F32, tag="cb")
                        nc.scalar.activation(cb[:], pt[:, :],
                                             mybir.ActivationFunctionType.Tanh,
                                             bias=bc[:])
                        nc.sync.dma_start(cbufd[64 * h:64 * h + 64, cols], cb[:])

        def final():
            """new_state = c + u*(state - c), natural layout, batched per
            (b, half). u/c come back [64, 1024]; state fp32 re-read."""
            stfv = stfd.rearrange("(i p) c -> p i c", p=128)
            outv = outd.rearrange("(i p) c -> p i c", p=128)
            for b in range(BL):
                for h in range(2):
                    i0 = h * 8
                    ut = small.tile([U, 1024], F32, tag="ut")
                    nc.sync.dma_start(
                        ut[:], ubufd[64 * h:64 * h + 64,
                                     b * 1024:(b + 1) * 1024])
                    ct = small.tile([U, 1024], F32, tag="ct")
                    nc.sync.dma_start(
                        ct[:], cbufd[64 * h:64 * h + 64,
                                     b * 1024:(b + 1) * 1024])
                    stt = small.tile([128, 512], F32, tag="stt")
                    nc.sync.dma_start(
                        stt[:].rearrange("p (i c) -> p i c", c=U),
                        stfv[:, i0:i0 + 8, b * U:(b + 1) * U])
                    cpt = psum.tile([128, 512], F32, tag="ps")
                    upt = psum.tile([128, 512], F32, tag="ps")
                    for j in range(8):
                        nc.tensor.transpose(cpt[:, j * U:(j + 1) * U],
                                            ct[:, j * 128:(j + 1) * 128],
                                            ident[:U, :U])
                        nc.tensor.transpose(upt[:, j * U:(j + 1) * U],
                                            ut[:, j * 128:(j + 1) * 128],
                                            ident[:U, :U])
                    # stt = (stt - c) * u + c, all [128, 512], in place
                    nc.vector.tensor_sub(stt[:], stt[:], cpt[:])
                    nc.vector.tensor_mul(stt[:], stt[:], upt[:])
                    nc.vector.tensor_add(stt[:], stt[:], cpt[:])
                    nc.sync.dma_start(
                        outv[:, i0:i0 + 8, b * U:(b + 1) * U],
                        stt[:].rearrange("p (i c) -> p i c", c=U))

        # ---- gate dconv ----
        dconv(x0sb, x1sb, True)
        finalize_inputs()
        w_stage(True)
        # ---- candidate dconv (x0sb is now candX in its state cols) ----
        dconv(x0sb, x1sb, False)
        w_stage(False)
        final()


_NC_CACHE = {}


def _get_nc():
    if "nc" not in _NC_CACHE:
        _NC_CACHE["nc"] = _build_nc()
    return _NC_CACHE["nc"]


def _host_prep(inputs, state, edges1, vals1, edges2, vals2, W_gate, b_gate,
               W_cand, b_cand):
    import ml_dtypes
    BF = ml_dtypes.bfloat16
    inputs = np.asarray(inputs, np.float32)
    state = np.asarray(state, np.float32)

    def densify_T(edges, vals):
        ST = np.zeros((N, N), np.float32)
        np.add.at(ST, (np.asarray(edges[1]).astype(np.int64),
                       np.asarray(edges[0]).astype(np.int64)),
                  np.asarray(vals, np.float32))
        return ST.astype(BF)

    SaT = densify_T(edges1, vals1)
    SbT = densify_T(edges2, vals2)

    def reorder(Wmat):
        Wmat = np.asarray(Wmat, np.float32)
        O = Wmat.shape[1]
        Wm = Wmat.reshape(F, M, O).copy()
        Wm[:, 2, :] *= 2.0
        Wm[:, 4, :] *= 2.0
        # state rows duplicated at partition bases 0 and 64
        Ws = np.ascontiguousarray(Wm[D_IN:].reshape(U, M * O))
        Ws2 = np.concatenate([Ws, Ws], 0)                       # [128, M*O]
        # input rows (m, fi) packed [10, O], replicated at bases 0/32/64
        Wi = np.ascontiguousarray(Wm[:D_IN].transpose(1, 0, 2).reshape(PK, O))
        Wi2 = np.zeros((128, O), np.float32)
        for base in (0, 32, 64):
            Wi2[base:base + PK] = Wi
        return (Ws2.astype(BF), Wi2.astype(BF))

    wgs, wgi = reorder(W_gate)
    wcs, wci = reorder(W_cand)
    bg = np.asarray(b_gate, np.float32).reshape(128, 1)
    bc = np.asarray(b_cand, np.float32).reshape(U, 1)

    in_maps = []
    for c in range(NCORES):
        bsl = slice(c * BL, (c + 1) * BL)
        st_c = state[bsl].reshape(BL, N, U)
        in_c = inputs[bsl].reshape(BL, N, D_IN)
        statef = np.ascontiguousarray(st_c.transpose(1, 0, 2).reshape(N, SC))
        x0 = np.empty((N, CW), np.float32)
        x0[:, :SC] = statef
        x0[:, SC:] = in_c.transpose(1, 0, 2).reshape(N, IC)
        in_maps.append(dict(x0=x0.astype(BF), statef=statef, sat=SaT,
                            sbt=SbT, wgs=wgs, wgi=wgi, wcs=wcs, wci=wci,
                            bg=bg, bc=bc))
    return in_maps


def kernel(**inputs):
    nc = _get_nc()
    in_maps = _host_prep(**inputs)
    res = run_bass_kernel_spmd(nc, in_maps, list(range(NCORES)))
    outs = []
    for c in range(NCORES):
        o = np.asarray(res.results[c]["out"])          # [N, (b, u)]
        outs.append(o.reshape(N, BL, U).transpose(1, 0, 2).reshape(BL, N * U))
    return np.concatenate(outs, 0).astype(np.float32)



# revision 14
# speedup vs baseline: 2.4726x; 2.4726x over previous
"""DCGRU cell on 8 Trainium2 NeuronCores (data-parallel over batch).

Design (v1, feature-major + fp8 DoubleRow):
  - All diffusion terms are direct functions of x0: with the Chebyshev
    fold x2 = 2*S^2 x0 - x0, host precomputes S^T and (S^2)^T per
    support and folds the constants into the projection weights
    (W0' = W0 - W2 - W4, W2' = 2 W2, W4' = 2 W4).  No chained spmm.
  - spmm runs feature-major: stationary = x0 natural node-major blocks,
    moving = S^T column chunks.  Output y^T = (S x)^T lands directly in
    the (batch,feature)-partition layout the projection consumes, so no
    PE transposes of diffusion outputs are needed.
  - Diffusion matmuls are fp8e4m3 with DoubleRow perf mode (256-node
    contraction per instruction, 0.5 cycles/row).  Each S matrix is
    pre-scaled by a power of two into fp8's normal range (S^2 is
    otherwise entirely subnormal); the inverse scale is folded into the
    bf16 projection weights.  Simulated end-to-end rel err: 3.4e-3.
  - Projection stays bf16: stationaries are 2-batch block-diagonal W
    tiles; the (m, input-feature) terms contract via an 80-partition
    packed xin^T tile shared by both dconvs.
  - Gate outputs stay feature-major: u^T kept in SBUF, candidate input
    candX^T = sigmoid(r)^T * state^T built feature-major; 64 small
    transposes produce the fp8 node-major candX stationary.  The final
    GRU combine runs feature-major and the host un-transposes.
"""

import numpy as np

import concourse.bass as bass
from concourse import bacc
import concourse.mybir as mybir
import concourse.tile as tile
from concourse.bass_utils import run_bass_kernel_spmd
from concourse.masks import make_identity

N = 2048            # nodes
B = 64              # global batch
BL = 8              # batch per core
NCORES = 8
D_IN = 2
U = 64              # hidden units
M = 5               # 1 + 2 supports * 2 steps
F = D_IN + U        # 66
NB = N // 128       # 16 node blocks
SC = BL * U         # 512 state cols in natural layout
IC = BL * D_IN      # 16 input cols
CW = SC + IC        # 528 natural cols per node block
NCH = 512           # node chunk (psum free size)
NC4 = N // NCH      # 4 chunks
J = BL // 2         # 4 batch pairs
PKM = 16            # packed input rows per m (8 b * 2 fi)

F32 = mybir.dt.float32
BF16 = mybir.dt.bfloat16
FP8 = mybir.dt.float8e4
DR = mybir.MatmulPerfMode.DoubleRow


def _build_nc():
    nc = bacc.Bacc(None, target_bir_lowering=False)

    x0d = nc.declare_dram_parameter("x0", [N, CW], FP8, isOutput=False)
    cxd = nc.declare_dram_parameter("cx", [N, CW], FP8, isOutput=False)
    stTd = nc.declare_dram_parameter("stT", [128, J * N], BF16, isOutput=False)
    xin0d = nc.declare_dram_parameter("xin0", [PKM, N], BF16, isOutput=False)
    sd = [nc.declare_dram_parameter(f"s{m}", [N, N], FP8, isOutput=False)
          for m in range(1, 5)]
    wgsd = nc.declare_dram_parameter("wgs", [128, 10 * 128], BF16, isOutput=False)
    wcsd = nc.declare_dram_parameter("wcs", [128, 5 * 128], BF16, isOutput=False)
    wigd = nc.declare_dram_parameter("wig", [128, 8 * 128], BF16, isOutput=False)
    wicd = nc.declare_dram_parameter("wic", [128, 4 * 128], BF16, isOutput=False)
    wig0d = nc.declare_dram_parameter("wig0", [PKM, 8 * 128], BF16, isOutput=False)
    wic0d = nc.declare_dram_parameter("wic0", [PKM, 4 * 128], BF16, isOutput=False)
    bgd = nc.declare_dram_parameter("bg", [128, 2], F32, isOutput=False)
    bcd = nc.declare_dram_parameter("bc", [128, 1], F32, isOutput=False)
    outd = nc.declare_dram_parameter("out", [128, J * N], BF16, isOutput=True)

    with tile.TileContext(nc) as tc:
        _emit(nc, tc, x0d, cxd, stTd, xin0d, sd, wgsd, wcsd, wigd, wicd,
              wig0d, wic0d, bgd, bcd, outd)
    nc.compile()
    return nc


def _emit(nc, tc, x0d, cxd, stTd, xin0d, sd, wgsd, wcsd, wigd, wicd,
          wig0d, wic0d, bgd, bcd, outd):
    from contextlib import ExitStack
    ctx = ExitStack()
    with ctx:
        consts = ctx.enter_context(tc.tile_pool(name="consts", bufs=1))
        acts = ctx.enter_context(tc.tile_pool(name="acts", bufs=1))
        spool = ctx.enter_context(tc.tile_pool(name="spool", bufs=3))
        small = ctx.enter_context(tc.tile_pool(name="small", bufs=3))
        psum = ctx.enter_context(tc.tile_pool(name="psum", bufs=8, space="PSUM"))

        ident = consts.tile([128, 128], F32)
        make_identity(nc, ident[:])
        identb = consts.tile([128, 128], BF16)
        nc.vector.tensor_copy(identb[:], ident[:])

        wgs = consts.tile([128, 10 * 128], BF16)
        wcs = consts.tile([128, 5 * 128], BF16)
        wig = consts.tile([128, 8 * 128], BF16)
        wic = consts.tile([128, 4 * 128], BF16)
        wig0 = consts.tile([PKM, 8 * 128], BF16)
        wic0 = consts.tile([PKM, 4 * 128], BF16)
        bg = consts.tile([128, 2], F32)
        bc = consts.tile([128, 1], F32)
        for dst, src in ((wgs, wgsd), (wcs, wcsd), (wig, wigd), (wic, wicd),
                         (wig0, wig0d), (wic0, wic0d), (bg, bgd), (bc, bcd)):
            nc.sync.dma_start(dst[:], src[:])

        # activations
        x0n = acts.tile([128, NB * CW], FP8, tag="x0n")      # natural x0
        cxn = acts.tile([128, NB * CW], FP8, tag="cxn")      # natural candX
        stT = acts.tile([128, J * N], BF16, tag="stT")       # state^T
        uT = acts.tile([128, J * N], BF16, tag="uT")
        cxT = acts.tile([128, J * N], BF16, tag="cxT")       # candX^T (state)
        # packed xin^T: m=1..4 at partition (m-1)*32 (rows 16..31 of each
        # group stay zero); m=0 host rows live in xinT0
        xinT = acts.tile([128, N], BF16, tag="xinT")
        xinT0 = acts.tile([PKM, N], BF16, tag="xinT0")
        nc.vector.memset(xinT[:], 0.0)
        xsT = acts.tile([128, 16 * NCH], BF16, tag="xsT")    # (m-1, j) chunk slices

        x0dv = x0d.rearrange("(t p) c -> t p c", p=128)
        cxdv = cxd.rearrange("(t p) c -> t p c", p=128)
        for i in range(NB):
            nc.sync.dma_start(x0n[:, i * CW:(i + 1) * CW], x0dv[i])
            nc.sync.dma_start(cxn[:, i * CW:(i + 1) * CW], cxdv[i])
        nc.sync.dma_start(stT[:], stTd[:])
        nc.sync.dma_start(xinT0[:], xin0d[:])

        sdv = [s.rearrange("(jb p) n -> p jb n", p=128) for s in sd]

        def xsT_s(m, j):
            return xsT[:, ((m - 1) * J + j) * NCH:((m - 1) * J + j + 1) * NCH]

        def diffuse(c, m, src, with_input):
            """y^T chunk c for S-matrix m (1..4); src = natural fp8 tile."""
            sc = spool.tile([128, NB * NCH], FP8, tag="sc")
            nc.sync.dma_start(
                sc[:].rearrange("p (jb n) -> p jb n", n=NCH),
                sdv[m - 1][:, :, c * NCH:(c + 1) * NCH])
            scv = sc[:].rearrange("p (jb n) -> p jb n", n=NCH)
            srcv = src[:].rearrange("p (t w) -> p t w", w=CW)
            ngroups = 5 if with_input else 4
            for g in range(ngroups):
                if g < 4:
                    pt = psum.tile([128, NCH], F32, tag="ps", name=f"pd{g}")
                    c0, c1 = g * 128, (g + 1) * 128
                else:
                    pt = psum.tile([PKM, NCH], F32, tag="ps", name="pdin")
                    c0, c1 = SC, SC + IC
                for t in range(8):
                    nc.tensor.matmul(
                        pt[:],
                        srcv[:, 2 * t:2 * t + 2, c0:c1],
                        scv[:, 2 * t:2 * t + 2, :],
                        start=(t == 0), stop=(t == 7), perf_mode=DR)
                if g < 4:
                    nc.vector.tensor_copy(xsT_s(m, g)[:], pt[:])
                else:
                    r0 = (m - 1) * 32
                    nc.vector.tensor_copy(
                        xinT[r0:r0 + PKM, c * NCH:(c + 1) * NCH], pt[:])

        def gate_proj(c):
            for j in range(J):
                stTs = stT[:, j * N + c * NCH:j * N + (c + 1) * NCH]
                for h in range(2):
                    pp = psum.tile([128, NCH], F32, tag="ps", name="pproj")
                    nc.tensor.matmul(pp[:], wgs[:, h * 128:(h + 1) * 128],
                                     stTs, start=True, stop=False)
                    for m in range(1, 5):
                        nc.tensor.matmul(
                            pp[:], wgs[:, (2 * m + h) * 128:(2 * m + h + 1) * 128],
                            xsT_s(m, j), start=False, stop=False)
                    nc.tensor.matmul(
                        pp[:], wig[:, (2 * j + h) * 128:(2 * j + h + 1) * 128],
                        xinT[:, c * NCH:(c + 1) * NCH],
                        start=False, stop=False)
                    nc.tensor.matmul(
                        pp[:], wig0[:, (2 * j + h) * 128:(2 * j + h + 1) * 128],
                        xinT0[:, c * NCH:(c + 1) * NCH],
                        start=False, stop=True)
                    if h == 0:
                        rT = small.tile([128, NCH], BF16, tag="rT")
                        nc.scalar.activation(
                            rT[:], pp[:],
                            mybir.ActivationFunctionType.Sigmoid,
                            bias=bg[:, 0:1])
                        nc.vector.tensor_mul(
                            cxT[:, j * N + c * NCH:j * N + (c + 1) * NCH],
                            rT[:], stTs)
                    else:
                        nc.scalar.activation(
                            uT[:, j * N + c * NCH:j * N + (c + 1) * NCH], pp[:],
                            mybir.ActivationFunctionType.Sigmoid,
                            bias=bg[:, 1:2])

        def candx_nat(c):
            """Transpose candX^T chunk back to natural fp8 stationary."""
            for j in range(J):
                tp = psum.tile([128, NCH], BF16, tag="ps", name="ptr")
                for nb in range(4):
                    nc.tensor.transpose(
                        tp[:, nb * 128:(nb + 1) * 128],
                        cxT[:, j * N + c * NCH + nb * 128:
                            j * N + c * NCH + (nb + 1) * 128],
                        identb[:])
                for nb in range(4):
                    i = c * 4 + nb
                    nc.vector.tensor_copy(
                        cxn[:, i * CW + j * 128:i * CW + (j + 1) * 128],
                        tp[:, nb * 128:(nb + 1) * 128])

        def cand_proj(c):
            for j in range(J):
                stTs = stT[:, j * N + c * NCH:j * N + (c + 1) * NCH]
                pp = psum.tile([128, NCH], F32, tag="ps", name="pproj")
                nc.tensor.matmul(pp[:], wcs[:, 0:128],
                                 cxT[:, j * N + c * NCH:j * N + (c + 1) * NCH],
                                 start=True, stop=False)
                for m in range(1, 5):
                    nc.tensor.matmul(pp[:], wcs[:, m * 128:(m + 1) * 128],
                                     xsT_s(m, j), start=False, stop=False)
                nc.tensor.matmul(pp[:], wic[:, j * 128:(j + 1) * 128],
                                 xinT[:, c * NCH:(c + 1) * NCH],
                                 start=False, stop=False)
                nc.tensor.matmul(pp[:], wic0[:, j * 128:(j + 1) * 128],
                                 xinT0[:, c * NCH:(c + 1) * NCH],
                                 start=False, stop=True)
                cT = small.tile([128, NCH], BF16, tag="cT")
                nc.scalar.activation(cT[:], pp[:],
                                     mybir.ActivationFunctionType.Tanh,
                                     bias=bc[:])
                ot = small.tile([128, NCH], BF16, tag="ot")
                uTs = uT[:, j * N + c * NCH:j * N + (c + 1) * NCH]
                nc.vector.tensor_sub(ot[:], stTs, cT[:])
                nc.vector.tensor_mul(ot[:], ot[:], uTs)
                nc.vector.tensor_add(ot[:], ot[:], cT[:])
                nc.sync.dma_start(
                    outd[:, j * N + c * NCH:j * N + (c + 1) * NCH], ot[:])

        # ---- gate ----
        for c in range(NC4):
            for m in range(1, 5):
                diffuse(c, m, x0n, True)
            gate_proj(c)
            candx_nat(c)
        # ---- candidate ----
        for c in range(NC4):
            for m in range(1, 5):
                diffuse(c, m, cxn, False)
            cand_proj(c)


_NC_CACHE = {}


def _get_nc():
    if "nc" not in _NC_CACHE:
        _NC_CACHE["nc"] = _build_nc()
    return _NC_CACHE["nc"]


def _host_prep(inputs, state, edges1, vals1, edges2, vals2, W_gate, b_gate,
               W_cand, b_cand):
    import ml_dtypes
    BF = ml_dtypes.bfloat16
    # values kept <= 224 so encodings are identical under e4m3 and e4m3fn
    F8 = ml_dtypes.float8_e4m3
    inputs = np.asarray(inputs, np.float32)
    state = np.asarray(state, np.float32)

    def densify_T(edges, vals):
        ST = np.zeros((N, N), np.float32)
        np.add.at(ST, (np.asarray(edges[1]).astype(np.int64),
                       np.asarray(edges[0]).astype(np.int64)),
                  np.asarray(vals, np.float32))
        return ST

    SaT = densify_T(edges1, vals1)
    SbT = densify_T(edges2, vals2)
    Sa2T = SaT @ SaT
    Sb2T = SbT @ SbT
    smats, sscale = [], []
    for S in (SaT, Sa2T, SbT, Sb2T):
        s = 2.0 ** np.floor(np.log2(224.0 / np.abs(S).max()))
        smats.append((S * s).astype(F8))
        sscale.append(s)

    def fold(Wmat):
        Wm = np.asarray(Wmat, np.float32).reshape(F, M, -1).copy()
        Wl = [Wm[:, 0] - Wm[:, 2] - Wm[:, 4], Wm[:, 1], 2.0 * Wm[:, 2],
              Wm[:, 3], 2.0 * Wm[:, 4]]
        for m in range(1, 5):
            Wl[m] = Wl[m] / sscale[m - 1]
        return Wl

    def blockdiag2(Wst):
        O = Wst.shape[1]
        Z = np.zeros((128, 2 * O), np.float32)
        Z[:64, :O] = Wst
        Z[64:, O:] = Wst
        return Z

    Wgl = fold(W_gate)
    Wcl = fold(W_cand)
    # state stationaries: gate [128, (m*2+h)*128], cand [128, m*128]
    wgs = np.zeros((128, 10 * 128), np.float32)
    for m in range(5):
        bd = blockdiag2(Wgl[m][D_IN:])                    # [128, 256]
        for h in range(2):
            # po = (b', oo) with oo = 64h..64h+63
            blk = np.zeros((128, 128), np.float32)
            blk[:64, :64] = Wgl[m][D_IN:, 64 * h:64 * h + 64]
            blk[64:, 64:] = Wgl[m][D_IN:, 64 * h:64 * h + 64]
            wgs[:, (2 * m + h) * 128:(2 * m + h + 1) * 128] = blk
    wcs = np.zeros((128, 5 * 128), np.float32)
    for m in range(5):
        wcs[:, m * 128:(m + 1) * 128] = blockdiag2(Wcl[m][D_IN:])
    # input stationaries: m=1..4 at rows (m-1)*32 + b*2 + fi; m=0 in wi*0
    wig = np.zeros((128, 8 * 128), np.float32)
    wic = np.zeros((128, 4 * 128), np.float32)
    wig0 = np.zeros((PKM, 8 * 128), np.float32)
    wic0 = np.zeros((PKM, 4 * 128), np.float32)
    for j in range(J):
        for bb in range(2):
            b = 2 * j + bb
            rows0 = slice(b * 2, b * 2 + 2)
            for h in range(2):
                wig0[rows0, (2 * j + h) * 128 + bb * 64:
                     (2 * j + h) * 128 + bb * 64 + 64] = \
                    Wgl[0][:D_IN, 64 * h:64 * h + 64]
            wic0[rows0, j * 128 + bb * 64:j * 128 + bb * 64 + 64] = \
                Wcl[0][:D_IN, :]
            for m in range(1, 5):
                rows = slice((m - 1) * 32 + b * 2, (m - 1) * 32 + b * 2 + 2)
                for h in range(2):
                    wig[rows, (2 * j + h) * 128 + bb * 64:
                        (2 * j + h) * 128 + bb * 64 + 64] = \
                        Wgl[m][:D_IN, 64 * h:64 * h + 64]
                wic[rows, j * 128 + bb * 64:j * 128 + bb * 64 + 64] = \
                    Wcl[m][:D_IN, :]
    bgh = np.stack([np.tile(np.asarray(b_gate, np.float32)[:64], 2),
                    np.tile(np.asarray(b_gate, np.float32)[64:], 2)], 1)
    bcv = np.tile(np.asarray(b_cand, np.float32), 2).reshape(128, 1)

    in_maps = []
    for cc in range(NCORES):
        bsl = slice(cc * BL, (cc + 1) * BL)
        st_c = state[bsl].reshape(BL, N, U)
        in_c = inputs[bsl].reshape(BL, N, D_IN)
        x0 = np.empty((N, CW), np.float32)
        x0[:, :SC] = st_c.transpose(1, 0, 2).reshape(N, SC)
        x0[:, SC:] = in_c.transpose(1, 0, 2).reshape(N, IC)
        stT = st_c.reshape(J, 2, N, U).transpose(0, 1, 3, 2) \
            .reshape(J, 128, N).transpose(1, 0, 2).reshape(128, J * N)
        xin0 = in_c.transpose(0, 2, 1).reshape(PKM, N)
        m = dict(x0=x0.astype(F8), cx=x0.astype(F8),
                 stT=stT.astype(BF), xin0=xin0.astype(BF),
                 wgs=wgs.astype(BF), wcs=wcs.astype(BF),
                 wig=wig.astype(BF), wic=wic.astype(BF),
                 wig0=wig0.astype(BF), wic0=wic0.astype(BF),
                 bg=bgh, bc=bcv)
        for i, sm in enumerate(smats):
            m[f"s{i + 1}"] = sm
        in_maps.append(m)
    return in_maps


def _assemble(res):
    outs = []
    for cc in range(NCORES):
        o = np.asarray(res.results[cc]["out"]).astype(np.float32)  # [128, J*N]
        o = o.reshape(2, U, J, N)            # [b', f, j, n]
        o = o.transpose(2, 0, 3, 1)          # [j, b', n, f]
        outs.append(o.reshape(BL, N * U))
    return np.concatenate(outs, 0)


def kernel(**inputs):
    nc = _get_nc()
    in_maps = _host_prep(**inputs)
    res = run_bass_kernel_spmd(nc, in_maps, list(range(NCORES)))
    return _assemble(res)


# revision 23
# speedup vs baseline: 2.5940x; 1.0491x over previous
"""DCGRU cell on 8 Trainium2 NeuronCores (data-parallel over batch).

Design (v1, feature-major + fp8 DoubleRow):
  - All diffusion terms are direct functions of x0: with the Chebyshev
    fold x2 = 2*S^2 x0 - x0, host precomputes S^T and (S^2)^T per
    support and folds the constants into the projection weights
    (W0' = W0 - W2 - W4, W2' = 2 W2, W4' = 2 W4).  No chained spmm.
  - spmm runs feature-major: stationary = x0 natural node-major blocks,
    moving = S^T column chunks.  Output y^T = (S x)^T lands directly in
    the (batch,feature)-partition layout the projection consumes, so no
    PE transposes of diffusion outputs are needed.
  - Diffusion matmuls are fp8e4m3 with DoubleRow perf mode (256-node
    contraction per instruction, 0.5 cycles/row).  Each S matrix is
    pre-scaled by a power of two into fp8's normal range (S^2 is
    otherwise entirely subnormal); the inverse scale is folded into the
    bf16 projection weights.  Simulated end-to-end rel err: 3.4e-3.
  - Projection stays bf16: stationaries are 2-batch block-diagonal W
    tiles; the (m, input-feature) terms contract via an 80-partition
    packed xin^T tile shared by both dconvs.
  - Gate outputs stay feature-major: u^T kept in SBUF, candidate input
    candX^T = sigmoid(r)^T * state^T built feature-major; 64 small
    transposes produce the fp8 node-major candX stationary.  The final
    GRU combine runs feature-major and the host un-transposes.
"""

import numpy as np

import concourse.bass as bass
from concourse import bacc
import concourse.mybir as mybir
import concourse.tile as tile
from concourse.bass_utils import run_bass_kernel_spmd
from concourse.masks import make_identity

N = 2048            # nodes
B = 64              # global batch
BL = 8              # batch per core
NCORES = 8
D_IN = 2
U = 64              # hidden units
M = 5               # 1 + 2 supports * 2 steps
F = D_IN + U        # 66
NB = N // 128       # 16 node blocks
SC = BL * U         # 512 state cols in natural layout
IC = BL * D_IN      # 16 input cols
CW = SC + IC        # 528 natural cols per node block
NCH = 512           # node chunk (psum free size)
NC4 = N // NCH      # 4 chunks
J = BL // 2         # 4 batch pairs
PKM = 16            # packed input rows per m (8 b * 2 fi)

F32 = mybir.dt.float32
BF16 = mybir.dt.bfloat16
FP8 = mybir.dt.float8e4
DR = mybir.MatmulPerfMode.DoubleRow


def _build_nc():
    nc = bacc.Bacc(None, target_bir_lowering=False)

    x0d = nc.declare_dram_parameter("x0", [N, CW], FP8, isOutput=False)
    stTd = nc.declare_dram_parameter("stT", [128, J * N], BF16, isOutput=False)
    xin0d = nc.declare_dram_parameter("xin0", [PKM, N], BF16, isOutput=False)
    sd = [nc.declare_dram_parameter(f"s{m}", [N, N], FP8, isOutput=False)
          for m in range(1, 5)]
    wgsd = nc.declare_dram_parameter("wgs", [128, 10 * 128], BF16, isOutput=False)
    wcsd = nc.declare_dram_parameter("wcs", [128, 5 * 128], BF16, isOutput=False)
    wigd = nc.declare_dram_parameter("wig", [128, 8 * 128], BF16, isOutput=False)
    wicd = nc.declare_dram_parameter("wic", [128, 4 * 128], BF16, isOutput=False)
    wig0d = nc.declare_dram_parameter("wig0", [PKM, 8 * 128], BF16, isOutput=False)
    wic0d = nc.declare_dram_parameter("wic0", [PKM, 4 * 128], BF16, isOutput=False)
    bgd = nc.declare_dram_parameter("bg", [128, 2], F32, isOutput=False)
    bcd = nc.declare_dram_parameter("bc", [128, 1], F32, isOutput=False)
    outd = nc.declare_dram_parameter("out", [128, J * N], BF16, isOutput=True)

    with tile.TileContext(nc) as tc:
        _emit(nc, tc, x0d, stTd, xin0d, sd, wgsd, wcsd, wigd, wicd,
              wig0d, wic0d, bgd, bcd, outd)
    nc.compile()
    return nc


def _emit(nc, tc, x0d, stTd, xin0d, sd, wgsd, wcsd, wigd, wicd,
          wig0d, wic0d, bgd, bcd, outd):
    from contextlib import ExitStack
    ctx = ExitStack()
    with ctx:
        consts = ctx.enter_context(tc.tile_pool(name="consts", bufs=1))
        acts = ctx.enter_context(tc.tile_pool(name="acts", bufs=1))
        spool = ctx.enter_context(tc.tile_pool(name="spool", bufs=2))
        small = ctx.enter_context(tc.tile_pool(name="small", bufs=3))
        psum = ctx.enter_context(tc.tile_pool(name="psum", bufs=8, space="PSUM"))

        # x0 load goes first so diffusion can start immediately; everything
        # else trails it in the DMA queues.
        x0n = acts.tile([128, NB * CW], FP8, tag="x0n")
        x0dv = x0d.rearrange("(t p) c -> p t c", p=128)
        nc.sync.dma_start(x0n[:].rearrange("p (t c) -> p t c", c=CW), x0dv)

        ident = consts.tile([128, 128], F32)
        make_identity(nc, ident[:])
        identb = consts.tile([128, 128], BF16)
        nc.vector.tensor_copy(identb[:], ident[:])

        wgs = consts.tile([128, 10 * 128], BF16)
        wcs = consts.tile([128, 5 * 128], BF16)
        wig = consts.tile([128, 8 * 128], BF16)
        wic = consts.tile([128, 4 * 128], BF16)
        wig0 = consts.tile([PKM, 8 * 128], BF16)
        wic0 = consts.tile([PKM, 4 * 128], BF16)
        bg = consts.tile([128, 2], F32)
        bc = consts.tile([128, 1], F32)
        for dst, src in ((wgs, wgsd), (wcs, wcsd), (wig, wigd), (wic, wicd),
                         (wig0, wig0d), (wic0, wic0d), (bg, bgd), (bc, bcd)):
            nc.sync.dma_start(dst[:], src[:])

        # activations
        cxn = acts.tile([128, NB * CW], FP8, tag="cxn")      # natural candX
        stT = acts.tile([128, J * N], BF16, tag="stT")       # state^T
        uT = acts.tile([128, J * N], BF16, tag="uT")
        cxT = acts.tile([128, J * N], BF16, tag="cxT")       # candX^T (state)
        # packed xin^T: m=1..4 at partition (m-1)*32 (rows 16..31 of each
        # group stay zero); m=0 host rows live in xinT0
        xinT = acts.tile([128, N], BF16, tag="xinT")
        xinT0 = acts.tile([PKM, N], BF16, tag="xinT0")
        nc.vector.memset(xinT[:], 0.0)
        xsT = acts.tile([128, 16 * NCH], BF16, tag="xsT")    # (m-1, j) chunk slices
        # resident S^T tiles for m=1 (S_a) and m=3 (S_b): loaded chunkwise
        # during the gate phase, reused without DMA in the candidate phase
        sres = {1: acts.tile([128, NB * N], FP8, tag="s1r", name="s1r"),
                3: acts.tile([128, NB * N], FP8, tag="s3r", name="s3r")}

        nc.sync.dma_start(stT[:], stTd[:])
        nc.sync.dma_start(xinT0[:], xin0d[:])

        sdv = [s.rearrange("(jb p) n -> p jb n", p=128) for s in sd]

        def xsT_s(m, j):
            return xsT[:, ((m - 1) * J + j) * NCH:((m - 1) * J + j + 1) * NCH]

        def diffuse(c, m, src, with_input, load):
            """y^T chunk c for S-matrix m (1..4); src = natural fp8 tile.
            m=1/3 use the resident tile (DMA'd on first pass only)."""
            if m in sres:
                scv = sres[m][:].rearrange(
                    "p (jb n) -> p jb n", n=N)[:, :, c * NCH:(c + 1) * NCH]
                if load:
                    nc.sync.dma_start(
                        scv, sdv[m - 1][:, :, c * NCH:(c + 1) * NCH])
            else:
                sc = spool.tile([128, NB * NCH], FP8, tag="sc")
                scv = sc[:].rearrange("p (jb n) -> p jb n", n=NCH)
                nc.sync.dma_start(
                    scv, sdv[m - 1][:, :, c * NCH:(c + 1) * NCH])
            srcv = src[:].rearrange("p (t w) -> p t w", w=CW)
            ngroups = 5 if with_input else 4
            for g in range(ngroups):
                if g < 4:
                    pt = psum.tile([128, NCH], F32, tag="ps", name=f"pd{g}")
                    c0, c1 = g * 128, (g + 1) * 128
                else:
                    pt = psum.tile([PKM, NCH], F32, tag="ps", name="pdin")
                    c0, c1 = SC, SC + IC
                for t in range(8):
                    nc.tensor.matmul(
                        pt[:],
                        srcv[:, 2 * t:2 * t + 2, c0:c1],
                        scv[:, 2 * t:2 * t + 2, :],
                        start=(t == 0), stop=(t == 7), perf_mode=DR)
                if g < 4:
                    nc.vector.tensor_copy(xsT_s(m, g)[:], pt[:])
                else:
                    r0 = (m - 1) * 32
                    nc.vector.tensor_copy(
                        xinT[r0:r0 + PKM, c * NCH:(c + 1) * NCH], pt[:])

        def gate_proj(c):
            for j in range(J):
                stTs = stT[:, j * N + c * NCH:j * N + (c + 1) * NCH]
                for h in range(2):
                    pp = psum.tile([128, NCH], F32, tag="ps", name="pproj")
                    nc.tensor.matmul(pp[:], wgs[:, h * 128:(h + 1) * 128],
                                     stTs, start=True, stop=False)
                    for m in range(1, 5):
                        nc.tensor.matmul(
                            pp[:], wgs[:, (2 * m + h) * 128:(2 * m + h + 1) * 128],
                            xsT_s(m, j), start=False, stop=False)
                    nc.tensor.matmul(
                        pp[:], wig[:, (2 * j + h) * 128:(2 * j + h + 1) * 128],
                        xinT[:, c * NCH:(c + 1) * NCH],
                        start=False, stop=False)
                    nc.tensor.matmul(
                        pp[:], wig0[:, (2 * j + h) * 128:(2 * j + h + 1) * 128],
                        xinT0[:, c * NCH:(c + 1) * NCH],
                        start=False, stop=True)
                    if h == 0:
                        rT = small.tile([128, NCH], BF16, tag="rT")
                        nc.scalar.activation(
                            rT[:], pp[:],
                            mybir.ActivationFunctionType.Sigmoid,
                            bias=bg[:, 0:1])
                        nc.vector.tensor_mul(
                            cxT[:, j * N + c * NCH:j * N + (c + 1) * NCH],
                            rT[:], stTs)
                    else:
                        nc.scalar.activation(
                            uT[:, j * N + c * NCH:j * N + (c + 1) * NCH], pp[:],
                            mybir.ActivationFunctionType.Sigmoid,
                            bias=bg[:, 1:2])

        def candx_nat(c):
            """Transpose candX^T chunk back to natural fp8 stationary."""
            for j in range(J):
                tp = psum.tile([128, NCH], BF16, tag="ps", name="ptr")
                for nb in range(4):
                    nc.tensor.transpose(
                        tp[:, nb * 128:(nb + 1) * 128],
                        cxT[:, j * N + c * NCH + nb * 128:
                            j * N + c * NCH + (nb + 1) * 128],
                        identb[:])
                for nb in range(4):
                    i = c * 4 + nb
                    nc.vector.tensor_copy(
                        cxn[:, i * CW + j * 128:i * CW + (j + 1) * 128],
                        tp[:, nb * 128:(nb + 1) * 128])

        def cand_proj(c):
            for j in range(J):
                stTs = stT[:, j * N + c * NCH:j * N + (c + 1) * NCH]
                pp = psum.tile([128, NCH], F32, tag="ps", name="pproj")
                nc.tensor.matmul(pp[:], wcs[:, 0:128],
                                 cxT[:, j * N + c * NCH:j * N + (c + 1) * NCH],
                                 start=True, stop=False)
                for m in range(1, 5):
                    nc.tensor.matmul(pp[:], wcs[:, m * 128:(m + 1) * 128],
                                     xsT_s(m, j), start=False, stop=False)
                nc.tensor.matmul(pp[:], wic[:, j * 128:(j + 1) * 128],
                                 xinT[:, c * NCH:(c + 1) * NCH],
                                 start=False, stop=False)
                nc.tensor.matmul(pp[:], wic0[:, j * 128:(j + 1) * 128],
                                 xinT0[:, c * NCH:(c + 1) * NCH],
                                 start=False, stop=True)
                cT = small.tile([128, NCH], BF16, tag="cT")
                nc.scalar.activation(cT[:], pp[:],
                                     mybir.ActivationFunctionType.Tanh,
                                     bias=bc[:])
                ot = small.tile([128, NCH], BF16, tag="ot")
                uTs = uT[:, j * N + c * NCH:j * N + (c + 1) * NCH]
                nc.vector.tensor_sub(ot[:], stTs, cT[:])
                nc.vector.tensor_mul(ot[:], ot[:], uTs)
                nc.vector.tensor_add(ot[:], ot[:], cT[:])
                nc.sync.dma_start(
                    outd[:, j * N + c * NCH:j * N + (c + 1) * NCH], ot[:])

        # ---- gate ----
        for c in range(NC4):
            for m in range(1, 5):
                diffuse(c, m, x0n, True, load=True)
            gate_proj(c)
            candx_nat(c)
        # ---- candidate ----
        for c in range(NC4):
            for m in range(1, 5):
                diffuse(c, m, cxn, False, load=False)
            cand_proj(c)


_NC_CACHE = {}


def _get_nc():
    if "nc" not in _NC_CACHE:
        _NC_CACHE["nc"] = _build_nc()
    return _NC_CACHE["nc"]


def _host_prep(inputs, state, edges1, vals1, edges2, vals2, W_gate, b_gate,
               W_cand, b_cand):
    import ml_dtypes
    BF = ml_dtypes.bfloat16
    # values kept <= 224 so encodings are identical under e4m3 and e4m3fn
    F8 = ml_dtypes.float8_e4m3
    inputs = np.asarray(inputs, np.float32)
    state = np.asarray(state, np.float32)

    def densify_T(edges, vals):
        ST = np.zeros((N, N), np.float32)
        np.add.at(ST, (np.asarray(edges[1]).astype(np.int64),
                       np.asarray(edges[0]).astype(np.int64)),
                  np.asarray(vals, np.float32))
        return ST

    SaT = densify_T(edges1, vals1)
    SbT = densify_T(edges2, vals2)
    Sa2T = SaT @ SaT
    Sb2T = SbT @ SbT
    smats, sscale = [], []
    for S in (SaT, Sa2T, SbT, Sb2T):
        s = 2.0 ** np.floor(np.log2(224.0 / np.abs(S).max()))
        smats.append((S * s).astype(F8))
        sscale.append(s)

    def fold(Wmat):
        Wm = np.asarray(Wmat, np.float32).reshape(F, M, -1).copy()
        Wl = [Wm[:, 0] - Wm[:, 2] - Wm[:, 4], Wm[:, 1], 2.0 * Wm[:, 2],
              Wm[:, 3], 2.0 * Wm[:, 4]]
        for m in range(1, 5):
            Wl[m] = Wl[m] / sscale[m - 1]
        return Wl

    def blockdiag2(Wst):
        O = Wst.shape[1]
        Z = np.zeros((128, 2 * O), np.float32)
        Z[:64, :O] = Wst
        Z[64:, O:] = Wst
        return Z

    Wgl = fold(W_gate)
    Wcl = fold(W_cand)
    # state stationaries: gate [128, (m*2+h)*128], cand [128, m*128]
    wgs = np.zeros((128, 10 * 128), np.float32)
    for m in range(5):
        bd = blockdiag2(Wgl[m][D_IN:])                    # [128, 256]
        for h in range(2):
            # po = (b', oo) with oo = 64h..64h+63
            blk = np.zeros((128, 128), np.float32)
            blk[:64, :64] = Wgl[m][D_IN:, 64 * h:64 * h + 64]
            blk[64:, 64:] = Wgl[m][D_IN:, 64 * h:64 * h + 64]
            wgs[:, (2 * m + h) * 128:(2 * m + h + 1) * 128] = blk
    wcs = np.zeros((128, 5 * 128), np.float32)
    for m in range(5):
        wcs[:, m * 128:(m + 1) * 128] = blockdiag2(Wcl[m][D_IN:])
    # input stationaries: m=1..4 at rows (m-1)*32 + b*2 + fi; m=0 in wi*0
    wig = np.zeros((128, 8 * 128), np.float32)
    wic = np.zeros((128, 4 * 128), np.float32)
    wig0 = np.zeros((PKM, 8 * 128), np.float32)
    wic0 = np.zeros((PKM, 4 * 128), np.float32)
    for j in range(J):
        for bb in range(2):
            b = 2 * j + bb
            rows0 = slice(b * 2, b * 2 + 2)
            for h in range(2):
                wig0[rows0, (2 * j + h) * 128 + bb * 64:
                     (2 * j + h) * 128 + bb * 64 + 64] = \
                    Wgl[0][:D_IN, 64 * h:64 * h + 64]
            wic0[rows0, j * 128 + bb * 64:j * 128 + bb * 64 + 64] = \
                Wcl[0][:D_IN, :]
            for m in range(1, 5):
                rows = slice((m - 1) * 32 + b * 2, (m - 1) * 32 + b * 2 + 2)
                for h in range(2):
                    wig[rows, (2 * j + h) * 128 + bb * 64:
                        (2 * j + h) * 128 + bb * 64 + 64] = \
                        Wgl[m][:D_IN, 64 * h:64 * h + 64]
                wic[rows, j * 128 + bb * 64:j * 128 + bb * 64 + 64] = \
                    Wcl[m][:D_IN, :]
    bgh = np.stack([np.tile(np.asarray(b_gate, np.float32)[:64], 2),
                    np.tile(np.asarray(b_gate, np.float32)[64:], 2)], 1)
    bcv = np.tile(np.asarray(b_cand, np.float32), 2).reshape(128, 1)

    in_maps = []
    for cc in range(NCORES):
        bsl = slice(cc * BL, (cc + 1) * BL)
        st_c = state[bsl].reshape(BL, N, U)
        in_c = inputs[bsl].reshape(BL, N, D_IN)
        x0 = np.empty((N, CW), np.float32)
        x0[:, :SC] = st_c.transpose(1, 0, 2).reshape(N, SC)
        x0[:, SC:] = in_c.transpose(1, 0, 2).reshape(N, IC)
        stT = st_c.reshape(J, 2, N, U).transpose(0, 1, 3, 2) \
            .reshape(J, 128, N).transpose(1, 0, 2).reshape(128, J * N)
        xin0 = in_c.transpose(0, 2, 1).reshape(PKM, N)
        m = dict(x0=x0.astype(F8),
                 stT=stT.astype(BF), xin0=xin0.astype(BF),
                 wgs=wgs.astype(BF), wcs=wcs.astype(BF),
                 wig=wig.astype(BF), wic=wic.astype(BF),
                 wig0=wig0.astype(BF), wic0=wic0.astype(BF),
                 bg=bgh, bc=bcv)
        for i, sm in enumerate(smats):
            m[f"s{i + 1}"] = sm
        in_maps.append(m)
    return in_maps


def _assemble(res):
    outs = []
    for cc in range(NCORES):
        o = np.asarray(res.results[cc]["out"]).astype(np.float32)  # [128, J*N]
        o = o.reshape(2, U, J, N)            # [b', f, j, n]
        o = o.transpose(2, 0, 3, 1)          # [j, b', n, f]
        outs.append(o.reshape(BL, N * U))
    return np.concatenate(outs, 0)


def kernel(**inputs):
    nc = _get_nc()
    in_maps = _host_prep(**inputs)
    res = run_bass_kernel_spmd(nc, in_maps, list(range(NCORES)))
    return _assemble(res)


# revision 25
# speedup vs baseline: 3.0538x; 1.1773x over previous
"""DCGRU cell on 8 Trainium2 NeuronCores (data-parallel over batch).

Design (v1, feature-major + fp8 DoubleRow):
  - All diffusion terms are direct functions of x0: with the Chebyshev
    fold x2 = 2*S^2 x0 - x0, host precomputes S^T and (S^2)^T per
    support and folds the constants into the projection weights
    (W0' = W0 - W2 - W4, W2' = 2 W2, W4' = 2 W4).  No chained spmm.
  - spmm runs feature-major: stationary = x0 natural node-major blocks,
    moving = S^T column chunks.  Output y^T = (S x)^T lands directly in
    the (batch,feature)-partition layout the projection consumes, so no
    PE transposes of diffusion outputs are needed.
  - Diffusion matmuls are fp8e4m3 with DoubleRow perf mode (256-node
    contraction per instruction, 0.5 cycles/row).  Each S matrix is
    pre-scaled by a power of two into fp8's normal range (S^2 is
    otherwise entirely subnormal); the inverse scale is folded into the
    bf16 projection weights.  Simulated end-to-end rel err: 3.4e-3.
  - Projection stays bf16: stationaries are 2-batch block-diagonal W
    tiles; the (m, input-feature) terms contract via an 80-partition
    packed xin^T tile shared by both dconvs.
  - Gate outputs stay feature-major: u^T kept in SBUF, candidate input
    candX^T = sigmoid(r)^T * state^T built feature-major; 64 small
    transposes produce the fp8 node-major candX stationary.  The final
    GRU combine runs feature-major and the host un-transposes.
"""

import numpy as np

import concourse.bass as bass
from concourse import bacc
import concourse.mybir as mybir
import concourse.tile as tile
from concourse.bass_utils import run_bass_kernel_spmd
from concourse.masks import make_identity

N = 2048            # nodes
B = 64              # global batch
BL = 8              # batch per core
NCORES = 8
D_IN = 2
U = 64              # hidden units
M = 5               # 1 + 2 supports * 2 steps
F = D_IN + U        # 66
NB = N // 128       # 16 node blocks
SC = BL * U         # 512 state cols in natural layout
IC = BL * D_IN      # 16 input cols
CW = SC + IC        # 528 natural cols per node block
NCH = 512           # node chunk (psum free size)
NC4 = N // NCH      # 4 chunks
J = BL // 2         # 4 batch pairs
PKM = 16            # packed input rows per m (8 b * 2 fi)

F32 = mybir.dt.float32
BF16 = mybir.dt.bfloat16
FP8 = mybir.dt.float8e4
DR = mybir.MatmulPerfMode.DoubleRow


def _build_nc():
    nc = bacc.Bacc(None, target_bir_lowering=False)

    x0d = nc.declare_dram_parameter("x0", [N, CW], FP8, isOutput=False)
    stTd = nc.declare_dram_parameter("stT", [128, J * N], BF16, isOutput=False)
    xind = nc.declare_dram_parameter("xin", [5 * PKM, N], BF16, isOutput=False)
    sd = [nc.declare_dram_parameter(f"s{m}", [N, N], FP8, isOutput=False)
          for m in range(1, 5)]
    wgsd = nc.declare_dram_parameter("wgs", [128, 10 * 128], BF16, isOutput=False)
    wcsd = nc.declare_dram_parameter("wcs", [128, 5 * 128], BF16, isOutput=False)
    wigd = nc.declare_dram_parameter("wig", [5 * PKM, 8 * 128], BF16, isOutput=False)
    wicd = nc.declare_dram_parameter("wic", [5 * PKM, 4 * 128], BF16, isOutput=False)
    bgd = nc.declare_dram_parameter("bg", [128, 2], F32, isOutput=False)
    bcd = nc.declare_dram_parameter("bc", [128, 1], F32, isOutput=False)
    outd = nc.declare_dram_parameter("out", [128, J * N], BF16, isOutput=True)

    with tile.TileContext(nc) as tc:
        _emit(nc, tc, x0d, stTd, xind, sd, wgsd, wcsd, wigd, wicd,
              bgd, bcd, outd)
    nc.compile()
    return nc


def _emit(nc, tc, x0d, stTd, xind, sd, wgsd, wcsd, wigd, wicd,
          bgd, bcd, outd):
    from contextlib import ExitStack
    ctx = ExitStack()
    with ctx:
        consts = ctx.enter_context(tc.tile_pool(name="consts", bufs=1))
        acts = ctx.enter_context(tc.tile_pool(name="acts", bufs=1))
        spool = ctx.enter_context(tc.tile_pool(name="spool", bufs=2))
        small = ctx.enter_context(tc.tile_pool(name="small", bufs=3))
        psum = ctx.enter_context(tc.tile_pool(name="psum", bufs=8, space="PSUM"))

        # x0 load goes first so diffusion can start immediately; everything
        # else trails it in the DMA queues.
        x0n = acts.tile([128, NB * CW], FP8, tag="x0n")
        x0dv = x0d.rearrange("(t p) c -> p t c", p=128)
        nc.sync.dma_start(x0n[:].rearrange("p (t c) -> p t c", c=CW), x0dv)

        ident = consts.tile([128, 128], F32)
        make_identity(nc, ident[:])
        identb = consts.tile([128, 128], BF16)
        nc.vector.tensor_copy(identb[:], ident[:])

        wgs = consts.tile([128, 10 * 128], BF16)
        wcs = consts.tile([128, 5 * 128], BF16)
        wig = consts.tile([5 * PKM, 8 * 128], BF16)
        wic = consts.tile([5 * PKM, 4 * 128], BF16)
        bg = consts.tile([128, 2], F32)
        bc = consts.tile([128, 1], F32)

        def load_gate_consts():
            for dst, sr in ((wgs, wgsd), (wig, wigd), (bg, bgd)):
                nc.sync.dma_start(dst[:], sr[:])

        def load_cand_consts():
            for dst, sr in ((wcs, wcsd), (wic, wicd), (bc, bcd)):
                nc.sync.dma_start(dst[:], sr[:])

        # activations
        cxn = acts.tile([128, NB * CW], FP8, tag="cxn")      # natural candX
        stT = acts.tile([128, J * N], BF16, tag="stT")       # state^T
        uT = acts.tile([128, J * N], BF16, tag="uT")
        cxT = acts.tile([128, J * N], BF16, tag="cxT")       # candX^T (state)
        # packed xin^T [(m, b, fi), n], all five m host-precomputed
        xinT = acts.tile([5 * PKM, N], BF16, tag="xinT")
        xsT = acts.tile([128, 16 * NCH], BF16, tag="xsT")    # (m-1, j) chunk slices
        # resident S^T tiles for m=1 (S_a) and m=3 (S_b): loaded chunkwise
        # during the gate phase, reused without DMA in the candidate phase
        sres = {1: acts.tile([128, NB * N], FP8, tag="s1r", name="s1r"),
                3: acts.tile([128, NB * N], FP8, tag="s3r", name="s3r")}


        sdv = [s.rearrange("(jb p) n -> p jb n", p=128) for s in sd]

        def xsT_s(m, j):
            return xsT[:, ((m - 1) * J + j) * NCH:((m - 1) * J + j + 1) * NCH]

        def diffuse(c, m, src, load):
            """y^T chunk c for S-matrix m (1..4); src = natural fp8 tile.
            m=1/3 use the resident tile (DMA'd on first pass only)."""
            if m in sres:
                scv = sres[m][:].rearrange(
                    "p (jb n) -> p jb n", n=N)[:, :, c * NCH:(c + 1) * NCH]
                if load:
                    nc.sync.dma_start(
                        scv, sdv[m - 1][:, :, c * NCH:(c + 1) * NCH])
            else:
                sc = spool.tile([128, NB * NCH], FP8, tag="sc")
                scv = sc[:].rearrange("p (jb n) -> p jb n", n=NCH)
                nc.sync.dma_start(
                    scv, sdv[m - 1][:, :, c * NCH:(c + 1) * NCH])
            srcv = src[:].rearrange("p (t w) -> p t w", w=CW)
            for g in range(4):
                pt = psum.tile([128, NCH], F32, tag="ps", name=f"pd{g}")
                c0, c1 = g * 128, (g + 1) * 128
                for t in range(8):
                    nc.tensor.matmul(
                        pt[:],
                        srcv[:, 2 * t:2 * t + 2, c0:c1],
                        scv[:, 2 * t:2 * t + 2, :],
                        start=(t == 0), stop=(t == 7), perf_mode=DR)
                nc.vector.tensor_copy(xsT_s(m, g)[:], pt[:])

        def gate_proj(c):
            for j in range(J):
                stTs = stT[:, j * N + c * NCH:j * N + (c + 1) * NCH]
                for h in range(2):
                    pp = psum.tile([128, NCH], F32, tag="ps", name="pproj")
                    nc.tensor.matmul(pp[:], wgs[:, h * 128:(h + 1) * 128],
                                     stTs, start=True, stop=False)
                    for m in range(1, 5):
                        nc.tensor.matmul(
                            pp[:], wgs[:, (2 * m + h) * 128:(2 * m + h + 1) * 128],
                            xsT_s(m, j), start=False, stop=False)
                    nc.tensor.matmul(
                        pp[:], wig[:, (2 * j + h) * 128:(2 * j + h + 1) * 128],
                        xinT[:, c * NCH:(c + 1) * NCH],
                        start=False, stop=True)
                    if h == 0:
                        rT = small.tile([128, NCH], BF16, tag="rT")
                        nc.scalar.activation(
                            rT[:], pp[:],
                            mybir.ActivationFunctionType.Sigmoid,
                            bias=bg[:, 0:1])
                        nc.vector.tensor_mul(
                            cxT[:, j * N + c * NCH:j * N + (c + 1) * NCH],
                            rT[:], stTs)
                    else:
                        nc.scalar.activation(
                            uT[:, j * N + c * NCH:j * N + (c + 1) * NCH], pp[:],
                            mybir.ActivationFunctionType.Sigmoid,
                            bias=bg[:, 1:2])

        def candx_nat(c):
            """Transpose candX^T chunk back to natural fp8 stationary."""
            for j in range(J):
                tp = psum.tile([128, NCH], BF16, tag="ps", name="ptr")
                for nb in range(4):
                    nc.tensor.transpose(
                        tp[:, nb * 128:(nb + 1) * 128],
                        cxT[:, j * N + c * NCH + nb * 128:
                            j * N + c * NCH + (nb + 1) * 128],
                        identb[:])
                for nb in range(4):
                    i = c * 4 + nb
                    nc.vector.tensor_copy(
                        cxn[:, i * CW + j * 128:i * CW + (j + 1) * 128],
                        tp[:, nb * 128:(nb + 1) * 128])

        def cand_proj(c):
            for j in range(J):
                stTs = stT[:, j * N + c * NCH:j * N + (c + 1) * NCH]
                pp = psum.tile([128, NCH], F32, tag="ps", name="pproj")
                nc.tensor.matmul(pp[:], wcs[:, 0:128],
                                 cxT[:, j * N + c * NCH:j * N + (c + 1) * NCH],
                                 start=True, stop=False)
                for m in range(1, 5):
                    nc.tensor.matmul(pp[:], wcs[:, m * 128:(m + 1) * 128],
                                     xsT_s(m, j), start=False, stop=False)
                nc.tensor.matmul(pp[:], wic[:, j * 128:(j + 1) * 128],
                                 xinT[:, c * NCH:(c + 1) * NCH],
                                 start=False, stop=True)
                cT = small.tile([128, NCH], BF16, tag="cT")
                nc.scalar.activation(cT[:], pp[:],
                                     mybir.ActivationFunctionType.Tanh,
                                     bias=bc[:])
                ot = small.tile([128, NCH], BF16, tag="ot")
                uTs = uT[:, j * N + c * NCH:j * N + (c + 1) * NCH]
                nc.vector.tensor_sub(ot[:], stTs, cT[:])
                nc.vector.tensor_mul(ot[:], ot[:], uTs)
                nc.vector.tensor_add(ot[:], ot[:], cT[:])
                nc.sync.dma_start(
                    outd[:, j * N + c * NCH:j * N + (c + 1) * NCH], ot[:])

        # ---- gate ----
        for c in range(NC4):
            for m in range(1, 5):
                diffuse(c, m, x0n, load=True)
                if c == 0 and m == 1:
                    nc.sync.dma_start(stT[:], stTd[:])
                    nc.sync.dma_start(xinT[:], xind[:])
                if c == 0 and m == 2:
                    load_gate_consts()
                if c == 0 and m == 3:
                    load_cand_consts()
            gate_proj(c)
            candx_nat(c)
        # ---- candidate ----
        for c in range(NC4):
            for m in range(1, 5):
                diffuse(c, m, cxn, load=False)
            cand_proj(c)


_NC_CACHE = {}


def _get_nc():
    if "nc" not in _NC_CACHE:
        _NC_CACHE["nc"] = _build_nc()
    return _NC_CACHE["nc"]


def _host_prep(inputs, state, edges1, vals1, edges2, vals2, W_gate, b_gate,
               W_cand, b_cand):
    import ml_dtypes
    BF = ml_dtypes.bfloat16
    # values kept <= 224 so encodings are identical under e4m3 and e4m3fn
    F8 = ml_dtypes.float8_e4m3
    inputs = np.asarray(inputs, np.float32)
    state = np.asarray(state, np.float32)

    def densify_T(edges, vals):
        ST = np.zeros((N, N), np.float32)
        np.add.at(ST, (np.asarray(edges[1]).astype(np.int64),
                       np.asarray(edges[0]).astype(np.int64)),
                  np.asarray(vals, np.float32))
        return ST

    SaT = densify_T(edges1, vals1)
    SbT = densify_T(edges2, vals2)
    Sa2T = SaT @ SaT
    Sb2T = SbT @ SbT
    smats, sscale = [], []
    smatsT = [SaT, Sa2T, SbT, Sb2T]
    for S in smatsT:
        s = 2.0 ** np.floor(np.log2(224.0 / np.abs(S).max()))
        smats.append((S * s).astype(F8))
        sscale.append(s)

    def fold(Wmat):
        Wm = np.asarray(Wmat, np.float32).reshape(F, M, -1).copy()
        Wl = [Wm[:, 0] - Wm[:, 2] - Wm[:, 4], Wm[:, 1], 2.0 * Wm[:, 2],
              Wm[:, 3], 2.0 * Wm[:, 4]]
        Wli = [w[:D_IN].copy() for w in Wl]      # input rows, unscaled
        for m in range(1, 5):
            Wl[m] = Wl[m] / sscale[m - 1]        # state rows absorb 1/s_m
        return Wl, Wli

    def blockdiag2(Wst):
        O = Wst.shape[1]
        Z = np.zeros((128, 2 * O), np.float32)
        Z[:64, :O] = Wst
        Z[64:, O:] = Wst
        return Z

    Wgl, Wgli = fold(W_gate)
    Wcl, Wcli = fold(W_cand)
    # state stationaries: gate [128, (m*2+h)*128], cand [128, m*128]
    wgs = np.zeros((128, 10 * 128), np.float32)
    for m in range(5):
        bd = blockdiag2(Wgl[m][D_IN:])                    # [128, 256]
        for h in range(2):
            # po = (b', oo) with oo = 64h..64h+63
            blk = np.zeros((128, 128), np.float32)
            blk[:64, :64] = Wgl[m][D_IN:, 64 * h:64 * h + 64]
            blk[64:, 64:] = Wgl[m][D_IN:, 64 * h:64 * h + 64]
            wgs[:, (2 * m + h) * 128:(2 * m + h + 1) * 128] = blk
    wcs = np.zeros((128, 5 * 128), np.float32)
    for m in range(5):
        wcs[:, m * 128:(m + 1) * 128] = blockdiag2(Wcl[m][D_IN:])
    # input stationaries: rows m*16 + b*2 + fi, unscaled (xin exact on host)
    wig = np.zeros((5 * PKM, 8 * 128), np.float32)
    wic = np.zeros((5 * PKM, 4 * 128), np.float32)
    for j in range(J):
        for bb in range(2):
            b = 2 * j + bb
            for m in range(5):
                rows = slice(m * PKM + b * 2, m * PKM + b * 2 + 2)
                for h in range(2):
                    wig[rows, (2 * j + h) * 128 + bb * 64:
                        (2 * j + h) * 128 + bb * 64 + 64] = \
                        Wgli[m][:, 64 * h:64 * h + 64]
                wic[rows, j * 128 + bb * 64:j * 128 + bb * 64 + 64] = \
                    Wcli[m][:, :]
    bgh = np.stack([np.tile(np.asarray(b_gate, np.float32)[:64], 2),
                    np.tile(np.asarray(b_gate, np.float32)[64:], 2)], 1)
    bcv = np.tile(np.asarray(b_cand, np.float32), 2).reshape(128, 1)

    in_maps = []
    for cc in range(NCORES):
        bsl = slice(cc * BL, (cc + 1) * BL)
        st_c = state[bsl].reshape(BL, N, U)
        in_c = inputs[bsl].reshape(BL, N, D_IN)
        x0 = np.empty((N, CW), np.float32)
        x0[:, :SC] = st_c.transpose(1, 0, 2).reshape(N, SC)
        x0[:, SC:] = in_c.transpose(1, 0, 2).reshape(N, IC)
        stT = st_c.reshape(J, 2, N, U).transpose(0, 1, 3, 2) \
            .reshape(J, 128, N).transpose(1, 0, 2).reshape(128, J * N)
        # host input diffusion: xin_m = S_m @ x_in (exact fp32)
        xin_nat = in_c.transpose(1, 0, 2).reshape(N, IC)   # [n, (b, fi)]
        xin = np.empty((5 * PKM, N), np.float32)
        xin[:PKM] = xin_nat.T
        for m in range(1, 5):
            xin[m * PKM:(m + 1) * PKM] = (smatsT[m - 1].T @ xin_nat).T
        m = dict(x0=x0.astype(F8),
                 stT=stT.astype(BF), xin=xin.astype(BF),
                 wgs=wgs.astype(BF), wcs=wcs.astype(BF),
                 wig=wig.astype(BF), wic=wic.astype(BF),
                 bg=bgh, bc=bcv)
        for i, sm in enumerate(smats):
            m[f"s{i + 1}"] = sm
        in_maps.append(m)
    return in_maps


def _assemble(res):
    outs = []
    for cc in range(NCORES):
        o = np.asarray(res.results[cc]["out"]).astype(np.float32)  # [128, J*N]
        o = o.reshape(2, U, J, N)            # [b', f, j, n]
        o = o.transpose(2, 0, 3, 1)          # [j, b', n, f]
        outs.append(o.reshape(BL, N * U))
    return np.concatenate(outs, 0)


def kernel(**inputs):
    nc = _get_nc()
    in_maps = _host_prep(**inputs)
    res = run_bass_kernel_spmd(nc, in_maps, list(range(NCORES)))
    return _assemble(res)
